# revision 2
# baseline (speedup 1.0000x reference)
"""Bass kernel builder for nn_PolylineSubGraphLayer (segment_reduce).

Pipeline (transposed layout: features on partitions, rows on free dim):
  xT [128, R] -> mm1 (f32r) -> h PSUM -> Silu(+b1) -> hs SBUF
  -> mm2 -> o PSUM [64, F] -> (+b2) o_sb SBUF, o2 = o_sb^2
  -> colsum matmul -> [mu; E[o^2]] PSUM [2, F] -> narrow stats -> rstd, mu*rstd
  -> bcast matmul -> statsB [128, F] PSUM
  -> z = (o_sb * rstdB) - murstdB  (optionally * ln_w + ln_b)
  -> masked segmented scans (tensor_tensor_scan, add/max, -1e30 resets)
     prefix: chained across tiles; suffix: reversed APs + 64-col halo
  -> seg = max(pfx, sfx)
  -> sq = (lnseg)^2 -> colsum -> ss -> narrow rsqrt-ish -> rn -> bcast rnB
  -> out = lnseg * rnB -> DMA to outT [128, R]

Host pre-transposes x and post-transposes the output.
"""

import sys

sys.path.insert(0, "/opt/trn_rl_repo")

from contextlib import ExitStack

import numpy as np

import concourse.bass as bass
import concourse.tile as tile
from concourse import bacc, mybir

F32 = mybir.dt.float32
F32R = mybir.dt.float32r
NEG = -1e30
EPS = 1e-5

IN_F = 128
HID = 256
OUT_F = 64


USE_F32R = False


def r32(ap):
    return ap.bitcast(F32R) if USE_F32R else ap


def build_nc(R, F=384, use_silu=True, apply_affine=True):
    """R: rows per core (multiple of F). Returns nc."""
    assert R % F == 0
    ntiles = R // F
    HALO = 64

    nc = bacc.Bacc("TRN2", target_bir_lowering=False, detect_race_conditions=False)

    xT = nc.dram_tensor("xT", [IN_F, R], F32, kind="ExternalInput")
    # mask[t] = 0 if clusters[t]==clusters[t-1] else NEG; length R+HALO+1,
    # everything >= R_valid is NEG.
    mask = nc.dram_tensor("mask", [1, R + HALO + 1], F32, kind="ExternalInput")
    w1 = nc.dram_tensor("w1", [IN_F, HID], F32, kind="ExternalInput")
    b1 = nc.dram_tensor("b1", [128, 2], F32, kind="ExternalInput")
    w2a = nc.dram_tensor("w2a", [128, OUT_F], F32, kind="ExternalInput")
    w2b = nc.dram_tensor("w2b", [128, OUT_F], F32, kind="ExternalInput")
    b2 = nc.dram_tensor("b2", [OUT_F, 1], F32, kind="ExternalInput")
    lnw = nc.dram_tensor("lnw", [OUT_F, 1], F32, kind="ExternalInput")
    lnb = nc.dram_tensor("lnb", [OUT_F, 1], F32, kind="ExternalInput")
    cswd = nc.dram_tensor("cswd", [128, 33], F32, kind="ExternalInput")
    sbcd = nc.dram_tensor("sbcd", [2, 128], F32, kind="ExternalInput")
    outT = nc.dram_tensor("outT", [IN_F, R], F32, kind="ExternalOutput")

    with ExitStack() as ctx:
        tc = ctx.enter_context(tile.TileContext(nc))
        consts = ctx.enter_context(tc.tile_pool(name="consts", bufs=1))
        sb = ctx.enter_context(tc.tile_pool(name="sb", bufs=3))
        sb2 = ctx.enter_context(tc.tile_pool(name="sb2", bufs=2))
        ln_pool = ctx.enter_context(tc.tile_pool(name="lnp", bufs=4))
        # PSUM pools; bufs are per-tag.
        ps_h = ctx.enter_context(tc.tile_pool(name="ps_h", bufs=2, space="PSUM"))
        ps_cs = ctx.enter_context(tc.tile_pool(name="ps_cs", bufs=2, space="PSUM"))
        ps_bc = ctx.enter_context(tc.tile_pool(name="ps_bc", bufs=2, space="PSUM"))
        ps_mk = ctx.enter_context(tc.tile_pool(name="ps_mk", bufs=2, space="PSUM"))

        # ---- constants ----
        w1_t = consts.tile([IN_F, HID], F32)
        nc.sync.dma_start(out=w1_t, in_=w1[:, :])
        b1_t = consts.tile([128, 2], F32)
        nc.sync.dma_start(out=b1_t, in_=b1[:, :])
        w2a_t = consts.tile([128, OUT_F], F32)
        nc.sync.dma_start(out=w2a_t, in_=w2a[:, :])
        w2b_t = consts.tile([128, OUT_F], F32)
        nc.sync.dma_start(out=w2b_t, in_=w2b[:, :])
        b2_t = consts.tile([OUT_F, 1], F32)
        nc.sync.dma_start(out=b2_t, in_=b2[:, :])
        lnw_t = consts.tile([OUT_F, 1], F32)
        nc.sync.dma_start(out=lnw_t, in_=lnw[:, :])
        lnb_t = consts.tile([OUT_F, 1], F32)
        nc.sync.dma_start(out=lnb_t, in_=lnb[:, :])

        ones64 = consts.tile([1, 64], F32)
        nc.vector.memset(ones64, 1.0)
        ones128c = consts.tile([128, 1], F32)
        nc.vector.memset(ones128c, 1.0)
        ones128r = consts.tile([1, 128], F32)
        nc.vector.memset(ones128r, 1.0)
        # colsum lhsT [128, 33]: col0 = 1/64 on k<64 (mu), col32 = 1/64 on k>=64
        csw = consts.tile([128, 33], F32)
        nc.sync.dma_start(out=csw, in_=cswd[:, :])
        # stats bcast lhsT [2, 128]: row0 -> out partitions 0:64, row1 -> 64:128
        sbc = consts.tile([2, 128], F32)
        nc.sync.dma_start(out=sbc, in_=sbcd[:, :])
        neg_init = consts.tile([64, 1], F32)
        nc.vector.memset(neg_init, NEG)
        eps_t = consts.tile([128, 1], F32)
        nc.vector.memset(eps_t, EPS)

        act_silu = mybir.ActivationFunctionType.Silu
        act_sigmoid = mybir.ActivationFunctionType.Sigmoid

        prev_pfx = None
        state = {}

        def stage_a(i):
            t0 = i * F
            xt = sb.tile([IN_F, F], F32, tag="xt")
            nc.sync.dma_start(out=xt, in_=xT[:, t0 : t0 + F])
            m_sb = sb.tile([1, F + HALO + 1], F32, tag="m_sb")
            nc.sync.dma_start(out=m_sb, in_=mask[:, t0 : t0 + F + HALO + 1])

            h1 = ps_h.tile([128, F], F32, tag="h")
            nc.tensor.matmul(
                out=h1, lhsT=r32(w1_t[:, 0:128]), rhs=r32(xt), start=True, stop=True
            )
            h2 = ps_h.tile([128, F], F32, tag="h")
            nc.tensor.matmul(
                out=h2, lhsT=r32(w1_t[:, 128:256]), rhs=r32(xt), start=True, stop=True
            )
            hs1 = sb.tile([128, F], F32, tag="hs1")
            hs2 = sb.tile([128, F], F32, tag="hs2")
            if use_silu:
                nc.scalar.activation(
                    out=hs1, in_=h1, func=act_silu, bias=b1_t[:, 0:1], scale=1.0
                )
                nc.scalar.activation(
                    out=hs2, in_=h2, func=act_silu, bias=b1_t[:, 1:2], scale=1.0
                )
            else:
                hb1 = sb.tile([128, F], F32, tag="hb1")
                hb2 = sb.tile([128, F], F32, tag="hb2")
                nc.vector.tensor_scalar_add(out=hb1, in0=h1, scalar1=b1_t[:, 0:1])
                nc.vector.tensor_scalar_add(out=hb2, in0=h2, scalar1=b1_t[:, 1:2])
                nc.scalar.activation(out=hs1, in_=hb1, func=act_sigmoid)
                nc.scalar.activation(out=hs2, in_=hb2, func=act_sigmoid)
                nc.vector.tensor_mul(out=hs1, in0=hs1, in1=hb1)
                nc.vector.tensor_mul(out=hs2, in0=hs2, in1=hb2)

            cs = ps_cs.tile([128, F], F32, tag="cs")
            o_ps = cs[0:OUT_F, :]
            nc.tensor.matmul(
                out=o_ps, lhsT=r32(w2a_t), rhs=r32(hs1), start=True, stop=False
            )
            nc.tensor.matmul(
                out=o_ps, lhsT=r32(w2b_t), rhs=r32(hs2), start=False, stop=True
            )
            oo = sb.tile([128, F], F32, tag="oo")
            nc.scalar.activation(
                out=oo[0:OUT_F, :],
                in_=o_ps,
                func=mybir.ActivationFunctionType.Identity,
                bias=b2_t[:, 0:1],
            )
            nc.scalar.activation(
                out=oo[OUT_F:128, :],
                in_=oo[0:OUT_F, :],
                func=mybir.ActivationFunctionType.Square,
            )

            nc.tensor.matmul(
                out=cs[0:33, :], lhsT=r32(csw), rhs=r32(oo), start=True, stop=True
            )
            mu_sb = sb2.tile([1, F], F32, tag="mu_sb")
            nc.vector.tensor_copy(out=mu_sb, in_=cs[0:1, :])
            rs_t = sb2.tile([1, F], F32, tag="rs_t")
            mr_t = sb2.tile([1, F], F32, tag="mr_t")
            mu2 = sb2.tile([1, F], F32, tag="mu2")
            nc.vector.tensor_mul(out=mu2, in0=mu_sb, in1=mu_sb)
            ve = sb2.tile([1, F], F32, tag="ve")
            nc.vector.tensor_sub(out=ve, in0=cs[32:33, :], in1=mu2)
            se = sb2.tile([1, F], F32, tag="se")
            nc.scalar.activation(
                out=se,
                in_=ve,
                func=mybir.ActivationFunctionType.Sqrt,
                bias=eps_t[0:1, 0:1],
            )
            nc.vector.reciprocal(out=rs_t, in_=se)
            nc.vector.tensor_mul(out=mr_t, in0=mu_sb, in1=rs_t)
            statsB = ps_bc.tile([128, F], F32, tag="bc")
            nc.tensor.matmul(
                out=statsB[0:64, :], lhsT=r32(ones64), rhs=r32(rs_t),
                start=True, stop=True,
            )
            nc.tensor.matmul(
                out=statsB[64:128, :], lhsT=r32(ones64), rhs=r32(mr_t),
                start=True, stop=True,
            )

            lnseg = ln_pool.tile([128, F + HALO], F32, tag="lnseg")
            zt = sb.tile([OUT_F, F], F32, tag="zt")
            nc.vector.tensor_mul(out=zt, in0=oo[0:OUT_F, :], in1=statsB[0:OUT_F, :])
            if apply_affine:
                z2 = sb.tile([OUT_F, F], F32, tag="z2")
                nc.vector.tensor_sub(out=z2, in0=zt, in1=statsB[OUT_F:128, :])
                nc.vector.tensor_scalar(
                    out=lnseg[0:OUT_F, 0:F],
                    in0=z2,
                    scalar1=lnw_t[:, 0:1],
                    scalar2=lnb_t[:, 0:1],
                    op0=mybir.AluOpType.mult,
                    op1=mybir.AluOpType.add,
                )
            else:
                nc.vector.tensor_sub(
                    out=lnseg[0:OUT_F, 0:F], in0=zt, in1=statsB[OUT_F:128, :]
                )
            if i == ntiles - 1:
                nc.vector.memset(lnseg[0:OUT_F, F : F + HALO], 0.0)
            if "lnseg" in state:
                nc.vector.tensor_copy(
                    out=state["lnseg"][0:OUT_F, F : F + HALO],
                    in_=lnseg[0:OUT_F, 0:HALO],
                )

            maskB = ps_mk.tile([OUT_F, F + HALO + 1], F32, tag="mk")
            nc.tensor.matmul(
                out=maskB, lhsT=r32(ones64), rhs=r32(m_sb), start=True, stop=True
            )
            pfx = sb2.tile([OUT_F, F], F32, tag="pfx")
            init = (
                neg_init[:, 0:1]
                if state.get("pfx") is None
                else state["pfx"][:, F - 1 : F]
            )
            nc.vector.tensor_tensor_scan(
                out=pfx,
                data0=maskB[:, 0:F],
                data1=lnseg[0:OUT_F, 0:F],
                initial=init,
                op0=mybir.AluOpType.add,
                op1=mybir.AluOpType.max,
            )
            return {"t0": t0, "lnseg": lnseg, "maskB": maskB, "pfx": pfx, "cs": cs}

        def stage_b(st):
            t0, lnseg, maskB, pfx, cs = (
                st["t0"], st["lnseg"], st["maskB"], st["pfx"], st["cs"],
            )
            sfx = sb2.tile([OUT_F, F + HALO], F32, tag="sfx")
            nc.vector.tensor_tensor_scan(
                out=sfx[:, ::-1],
                data0=maskB[:, 1 : F + HALO + 1][:, ::-1],
                data1=lnseg[0:OUT_F, 0 : F + HALO][:, ::-1],
                initial=NEG,
                op0=mybir.AluOpType.add,
                op1=mybir.AluOpType.max,
            )
            nc.vector.tensor_max(out=lnseg[OUT_F:128, 0:F], in0=pfx, in1=sfx[:, 0:F])

            sq = sb.tile([128, F], F32, tag="sq")
            nc.scalar.activation(
                out=sq,
                in_=lnseg[:, 0:F],
                func=mybir.ActivationFunctionType.Square,
            )
            nc.tensor.matmul(
                out=cs[64:65, :], lhsT=r32(ones128c), rhs=r32(sq), start=True, stop=True
            )
            ss_sb = sb2.tile([1, F], F32, tag="ss_sb")
            nc.vector.tensor_copy(out=ss_sb, in_=cs[64:65, :])
            sn = sb2.tile([1, F], F32, tag="sn")
            nc.scalar.activation(
                out=sn, in_=ss_sb, func=mybir.ActivationFunctionType.Sqrt
            )
            nc.vector.tensor_scalar_max(out=sn, in0=sn, scalar1=1e-12)
            rn = sb2.tile([1, F], F32, tag="rn")
            nc.vector.reciprocal(out=rn, in_=sn)
            rnB = ps_bc.tile([128, F], F32, tag="bc")
            nc.tensor.matmul(
                out=rnB, lhsT=r32(ones128r), rhs=r32(rn), start=True, stop=True
            )
            out_sb = sb.tile([128, F], F32, tag="out_sb")
            nc.vector.tensor_mul(out=out_sb, in0=lnseg[:, 0:F], in1=rnB)
            nc.sync.dma_start(out=outT[:, t0 : t0 + F], in_=out_sb)

        for i in range(ntiles):
            new_state = stage_a(i)
            if state:
                stage_b(state)
            state = new_state
        stage_b(state)

    nc.compile()
    return nc


# ---------------- host-side helpers ----------------


def host_prepare(x, clusters, W1, b1v, W2, b2v, ln_w, ln_b, n_cores=8, F=384):
    """Shard rows across cores at cluster boundaries; build per-core inputs."""
    N = x.shape[0]
    cl = np.asarray(clusters)
    # candidate split points at multiples of N/n_cores, snapped to cluster starts
    bounds = [0]
    for c in range(1, n_cores):
        tgt = (N * c) // n_cores
        # first index >= tgt where cluster changes (cl sorted)
        v = cl[tgt]
        s = int(np.searchsorted(cl, v, side="left"))
        e = int(np.searchsorted(cl, v, side="right"))
        # snap to nearer boundary of the cluster containing tgt
        bounds.append(s if (tgt - s) <= (e - tgt) else e)
    bounds.append(N)
    bounds = sorted(set(bounds))
    while len(bounds) < n_cores + 1:
        bounds.append(N)
    sizes = [bounds[i + 1] - bounds[i] for i in range(n_cores)]
    R = max(sizes)
    R = ((R + F - 1) // F) * F

    HALO = 64
    b1r = np.zeros((128, 2), np.float32)
    b1r[:, 0] = b1v[0:128]
    b1r[:, 1] = b1v[128:256]

    csw_np = np.zeros((128, 33), np.float32)
    csw_np[0:64, 0] = 1.0 / 64
    csw_np[64:128, 32] = 1.0 / 64
    sbc_np = np.zeros((2, 128), np.float32)
    sbc_np[0, 0:64] = 1.0
    sbc_np[1, 64:128] = 1.0

    xT_full = np.ascontiguousarray(x.T.astype(np.float32))  # [128, N]
    # mask over full array
    m_full = np.full(N, NEG, np.float32)
    m_full[1:] = np.where(cl[1:] == cl[:-1], 0.0, NEG).astype(np.float32)

    in_maps = []
    for c in range(n_cores):
        s, e = bounds[c], bounds[c + 1]
        n = e - s
        xT = np.zeros((128, R), np.float32)
        xT[:, 0:n] = xT_full[:, s:e]
        mk = np.full((1, R + HALO + 1), NEG, np.float32)
        mk[0, 0:n] = m_full[s:e]
        mk[0, 0] = NEG  # segment start at shard head
        in_maps.append(
            {
                "xT": xT,
                "mask": mk,
                "w1": np.ascontiguousarray(W1.astype(np.float32)),
                "b1": b1r,
                "w2a": np.ascontiguousarray(W2[0:128].astype(np.float32)),
                "w2b": np.ascontiguousarray(W2[128:256].astype(np.float32)),
                "b2": b2v.reshape(64, 1).astype(np.float32),
                "lnw": ln_w.reshape(64, 1).astype(np.float32),
                "lnb": ln_b.reshape(64, 1).astype(np.float32),
                "cswd": csw_np,
                "sbcd": sbc_np,
            }
        )
    return in_maps, bounds, R


def host_gather(outTs, bounds, N):
    out = np.empty((N, 128), np.float32)
    for c in range(len(bounds) - 1):
        s, e = bounds[c], bounds[c + 1]
        out[s:e] = outTs[c][:, 0 : e - s].T
    return out


def ref_numpy(x, clusters, W1, b1v, W2, b2v, ln_w, ln_b):
    h = x @ W1 + b1v
    h = h / (1.0 + np.exp(-h)) * 1.0  # silu = h*sigmoid(h)
    o = h @ W2 + b2v
    mean = o.mean(-1, keepdims=True)
    var = o.var(-1, keepdims=True)
    o = (o - mean) / np.sqrt(var + EPS) * ln_w + ln_b
    C = int(clusters.max()) + 1
    aggr = np.full((C, o.shape[1]), -np.inf, np.float32)
    np.maximum.at(aggr, clusters, o)
    out = np.concatenate([o, aggr[clusters]], -1)
    nrm = np.linalg.norm(out, axis=-1, keepdims=True)
    return out / np.maximum(nrm, 1e-12)


# ---------------- self-contained kernel entry ----------------

_CACHE = {}


def _get_nc(R, F, trivial_affine):
    key = (R, F, trivial_affine)
    if key not in _CACHE:
        _CACHE[key] = build_nc(R, F=F, use_silu=True, apply_affine=not trivial_affine)
    return _CACHE[key]


LAST_EXEC_NS = None
TRACE = False


def _install_profile_shim():
    """Register the axon NTFF profile hook (antenv.axon_hooks is a stub in
    this container) and neuter the artifact upload."""
    import types

    import antenv
    from concourse import bass_utils as _bu

    if "antenv.axon_hooks" not in sys.modules:
        mod = types.ModuleType("antenv.axon_hooks")
        _h = [None]
        mod.set_axon_ntff_profile_hook = lambda h: _h.__setitem__(0, h)
        mod.get_axon_ntff_profile_hook = lambda: _h[0]
        sys.modules["antenv.axon_hooks"] = mod
        antenv.axon_hooks = mod
        from trn_agent_boot.trn_boot import _ntff_profile_via_ctypes

        mod.set_axon_ntff_profile_hook(
            _ntff_profile_via_ctypes("/opt/axon/libaxon_pjrt.so")
        )
    _bu.upload_artifacts = lambda tmpdir: ""


def kernel(x, clusters, batch, W1, b1, W2, b2, ln_w, ln_b):
    global LAST_EXEC_NS
    from concourse.bass_utils import run_bass_kernel_spmd

    x = np.asarray(x)
    clusters = np.asarray(clusters)
    N = x.shape[0]
    F = 384
    n_cores = 8
    in_maps, bounds, R = host_prepare(
        x, clusters, np.asarray(W1), np.asarray(b1), np.asarray(W2), np.asarray(b2),
        np.asarray(ln_w), np.asarray(ln_b), n_cores=n_cores, F=F,
    )
    trivial = bool(
        np.all(np.asarray(ln_w) == 1.0) and np.all(np.asarray(ln_b) == 0.0)
    )
    nc = _get_nc(R, F, trivial)
    if TRACE:
        _install_profile_shim()
    res = run_bass_kernel_spmd(
        nc, in_maps, core_ids=list(range(n_cores)), trace=TRACE
    )
    LAST_EXEC_NS = res.exec_time_ns
    outs = [res.results[c]["outT"] for c in range(n_cores)]
    return host_gather(outs, bounds, N)
